# revision 2
# baseline (speedup 1.0000x reference)
# Bass/Trainium2 kernel for nn_LoRARouter (topk_masking) — v2.
#
# Reference:
#   logits = pooled @ (Wr @ Wg)^T ;  probs = softmax(logits)
#   out[m,b,e] = probs[b,m] > 0.5 ? (rank<2)/2 : (rank<1)
#
# Sharding (v2, d-parallel):
#   Core i owns d-columns [512i, 512(i+1)) of Wg/Weff and computes
#   PARTIAL logits for ALL 8192 batches over its 512-column contraction
#   slice, so no collective gates the bulk matmul. One ReduceScatter(add)
#   of the partial logits ([64,1024] f32 -> [8,1024] per core) at the end
#   hands each core the summed logits for its batch shard; the cheap
#   softmax/top-k select tail runs after it.
#
#   - modules padded 7 -> 8 (zero row of Wr) for even/pow2 layouts
#   - all matmuls are orientation-A fp32 (stationary [128,8], moving wide)
#     -- the only PE mode measured to keep full fp32 accuracy -- and are
#     column-tiled across 3 PE tiles (tile_position=(0,32j)) for ~2.3x
#     concurrency, keeping total PE time under the HBM stream time.
#   - rank computation from rand_noise runs on DVE during the stream.

import numpy as np

import concourse.bass as bass
import concourse.bacc as bacc
import concourse.mybir as mybir
import concourse.tile as tile
from concourse.bass_utils import run_bass_kernel_spmd

F32 = mybir.dt.float32
N_CORES = 8
B, D, NM, NE = 8192, 4096, 7, 8      # batch, d_model, n_modules, n_experts
NMP = 8                              # modules padded to 8
BS = B // N_CORES                    # 1024 batch rows per core (output shard)
DS = D // N_CORES                    # 512 Weff columns per core (contraction shard)
NBC = BS // 128                      # 8 batch chunks of 128 for the select phase
NKE = D // 128                       # 32 e-chunks (weff contraction)
NKD = DS // 128                      # 4 d-chunks (logits contraction)
NSP = B // 512                       # 16 batch spans of 512
NPC = 4                              # xT pieces per d-chunk (4 spans each)
GRP = NM * NE                        # 56 select columns per batch chunk
W = NBC * GRP                        # 448
TILE_COLS = ((0, 172), (172, 342), (342, 512))  # weff col split across PE tiles

ALU = mybir.AluOpType
AF = mybir.ActivationFunctionType

_SPAN_PERM = np.concatenate(
    [np.arange(512 * s, 512 * (s + 1)) for s in
     list(range(0, 16, 2)) + list(range(1, 16, 2))]
)

_CACHE = {}
LAST_RESULTS = None  # test harness introspection


def _build_program():
    nc = bacc.Bacc(
        "TRN2", target_bir_lowering=False, debug=False, num_devices=N_CORES
    )

    # pooled^T d-shard for ALL batches: xT[p, b] = pooled[b, 512i + p]
    xT = nc.dram_tensor("xT", [DS, B], F32, kind="ExternalInput")
    # Wg column shard [4096, 512]
    wg = nc.dram_tensor("wg", [D, DS], F32, kind="ExternalInput")
    # padded Wr in e-chunk layout: wrt[p, k*8+m] = WrP[m, 128k+p]
    wrt = nc.dram_tensor("wrt", [128, NKE * NMP], F32, kind="ExternalInput")
    nzin = nc.dram_tensor("nz", [128, W], F32, kind="ExternalInput")
    cst = nc.dram_tensor("cst", [128, W], F32, kind="ExternalInput")
    # 8-way-sum selector: sel[8j+m, m] = 1 (stationary for the PE reduce)
    selin = nc.dram_tensor("sel", [N_CORES * NMP, NMP], F32, kind="ExternalInput")
    o = nc.dram_tensor("o", [128, W], F32, kind="ExternalOutput")

    # AllToAll bounce: block i of a2a_in (rows [8i, 8i+8)) = this core's
    # partial logits for batch shard i; after the exchange, block j of
    # a2a_out = core j's partial for THIS core's shard. The 8-way sum is
    # done on-device (AllToAll is forced onto the Mesh algorithm — the
    # RDH path ReduceScatter picks at this size returns corrupt data).
    HBS = BS // 2
    rs_in = [nc.dram_tensor(f"rs_in{h}", [N_CORES * NMP, HBS], F32)
             for h in range(2)]
    rs_out = [nc.dram_tensor(f"rs_out{h}", [N_CORES * NMP, HBS], F32)
              for h in range(2)]

    with tile.TileContext(nc) as tc:
        with (
            tc.tile_pool(name="big", bufs=1) as bp,
            tc.tile_pool(name="small", bufs=1) as sp,
            tc.tile_pool(name="scr", bufs=2) as scp,
            tc.tile_pool(name="sm", bufs=16) as smp,
            tc.tile_pool(name="ps", bufs=6, space="PSUM") as ps,
            tc.tile_pool(name="pt", bufs=2, space="PSUM") as pt,
        ):
            # ---- input DMAs (sync ring, FIFO: small -> wg -> xT) ----
            wrt_sb = sp.tile([128, NKE * NMP], F32, tag="wrt")
            nz = sp.tile([128, W], F32, tag="nz")
            cstt = sp.tile([128, W], F32, tag="cst")
            sel_sb = sp.tile([N_CORES * NMP, NMP], F32, tag="sel")
            nc.sync.dma_start(wrt_sb[:], wrt[:])
            nc.sync.dma_start(nz[:], nzin[:])
            nc.sync.dma_start(cstt[:], cst[:])
            nc.sync.dma_start(sel_sb[:], selin[:])

            ident = sp.tile([128, 128], F32, tag="ident")
            from concourse.masks import make_identity
            make_identity(nc, ident[:])

            # bulk stream on the sync ring (HBM-bound at ~330 GB/s; a
            # second ring does not add bandwidth): wg first, then the even
            # batch half (pieces g=0,1, all k), then the odd half — so the
            # first exchange triggers at mid-stream.
            wg_r = wg[:].rearrange("(k p) d -> k p d", p=128)
            wgt = []
            for k in range(NKE):
                t = bp.tile([128, DS], F32, tag="wg", bufs=8)
                nc.sync.dma_start(t[:], wg_r[k])
                wgt.append(t)

            xT_r = xT[:].rearrange("(k p) (g c) -> k g p c", p=128, c=4 * 512)
            xps = {}
            for gpair in ((0, 1), (2, 3)):
                for k in range(NKD):
                    for g in gpair:
                        t = bp.tile([128, 4 * 512], F32, tag="xp", bufs=16)
                        nc.sync.dma_start(t[:], xT_r[k, g])
                        xps[(k, g)] = t

            # ---- Weff shard = Wr @ Wg[:, dshard] -> [8, 512], full e
            # contraction on-core; col-tiled over 3 PE tiles. ----
            pw = ps.tile([128, DS], F32, tag="ps", name="pw")
            for k in range(NKE):
                for j, (c0, c1) in enumerate(TILE_COLS):
                    nc.tensor.matmul(
                        pw[32 * j:32 * j + NMP, c0:c1],
                        wrt_sb[:, k * NMP:(k + 1) * NMP],
                        wgt[k][:, c0:c1],
                        start=(k == 0), stop=(k == NKE - 1),
                        tile_position=(0, 32 * j),
                    )
            weff_sb = sp.tile([NMP, DS], F32, tag="weff")
            for j, (c0, c1) in enumerate(TILE_COLS):
                nc.vector.tensor_copy(
                    weff_sb[:, c0:c1], pw[32 * j:32 * j + NMP, c0:c1]
                )
            # transpose to logits stationaries wtT[p, c*8+m] = WeffT[128c+p, m]
            wtT = sp.tile([128, NKD * NMP], F32, tag="wtT")
            for c in range(NKD):
                tr = pt.tile([128, NMP], F32, tag="pt")
                nc.tensor.transpose(
                    tr[:], weff_sb[:, c * 128:(c + 1) * 128], ident[:NMP, :NMP]
                )
                nc.vector.tensor_copy(wtT[:, c * NMP:(c + 1) * NMP], tr[:])

            # ---- expert ranks from rand_noise (DVE; overlaps the stream).
            # r[e] = #{j<e: v_j >= v_e} + #{j>e: v_j > v_e}; acc starts at
            # cst[e] = 7-e, each offset's compare adds at e / subtracts at
            # e-off (same scheme as validated baseline). ----
            acc = sp.tile([128, W], F32, tag="acc")
            nc.vector.tensor_copy(acc[:], cstt[:])
            nz_r = nz[:].rearrange("p (c m e) -> p c m e", m=NM, e=NE)
            acc_r = acc[:].rearrange("p (c m e) -> p c m e", m=NM, e=NE)
            for off in range(1, NE):
                wdt = NE - off
                scr = scp.tile([128, NBC * NM * 7], F32, tag="scr")
                scr_v = scr[:, : NBC * NM * wdt].rearrange(
                    "p (c m e) -> p c m e", m=NM, e=wdt
                )
                nc.vector.tensor_tensor(
                    scr_v, nz_r[:, :, :, 0:wdt], nz_r[:, :, :, off:NE], ALU.is_ge
                )
                nc.vector.tensor_tensor(
                    acc_r[:, :, :, off:NE], acc_r[:, :, :, off:NE], scr_v, ALU.add
                )
                nc.vector.tensor_tensor(
                    acc_r[:, :, :, 0:wdt], acc_r[:, :, :, 0:wdt], scr_v, ALU.subtract
                )
            # branch masks, ready before the collective lands:
            #   A1 = (rank<1)           (prob <= 0.5 branch)
            #   delta = 0.5*(rank<2) - A1   so  out = A1 + cond*delta
            a1 = sp.tile([128, W], F32, tag="a1")
            dlt = sp.tile([128, W], F32, tag="dlt")
            nc.vector.tensor_scalar(
                out=a1[:], in0=acc[:], scalar1=1.0, scalar2=None, op0=ALU.is_lt
            )
            nc.vector.tensor_scalar(
                out=dlt[:], in0=acc[:], scalar1=2.0, scalar2=0.5,
                op0=ALU.is_lt, op1=ALU.mult,
            )
            nc.vector.tensor_tensor(dlt[:], dlt[:], a1[:], ALU.subtract)

            # ---- partial logits, col-tiled: span s -> PE tile s%3 at PSUM
            # partitions 32*(s%3) of bank s//3; accumulate over the 4
            # d-chunks (k outer so each xT piece is consumed as it lands).
            pls = [
                ps.tile([128, 512], F32, tag="ps", name=f"pl{b}")
                for b in range(6)
            ]
            # per half: matmuls -> evac (q = permuted span index; q<8 even
            # global spans, q>=8 odd) -> rs store -> AllToAll. Each half's
            # exchange triggers as soon as ITS spans finish, so the first
            # fires at mid-stream and absorbs the cross-core skew while the
            # second half still computes. Mesh algorithm (the RDH path
            # ReduceScatter picks at this size returns corrupt data).
            HBS = BS // 2
            # two evac target tiles (vector-written / scalar-written) so the
            # two engines' psum evacuations don't serialize on a shared
            # tile's write-dependency tracking
            logT_v = sp.tile([NMP, B // 2], F32, tag="logTv")
            logT_s = sp.tile([NMP, B // 2], F32, tag="logTs")
            for h, gpair in enumerate(((0, 1), (2, 3))):
                for k in range(NKD):
                    for g in gpair:
                        for sl in range(4):
                            s = 4 * g + sl
                            j = s % 3
                            nc.tensor.matmul(
                                pls[s // 3][32 * j:32 * j + NMP, :],
                                wtT[:, k * NMP:(k + 1) * NMP],
                                xps[(k, g)][:, sl * 512:(sl + 1) * 512],
                                start=(k == 0), stop=(k == NKD - 1),
                                tile_position=(0, 32 * j),
                            )
                # evac this half's spans; even-q -> vector -> logT_v,
                # odd-q -> scalar -> logT_s (disjoint tiles, no serializing)
                for q in range(8 * h, 8 * h + 8):
                    j = q % 3
                    srcp = pls[q // 3][32 * j:32 * j + NMP, :]
                    if q % 2 == 0:
                        nc.vector.tensor_copy(
                            logT_v[:, (q // 2) * 512:(q // 2 + 1) * 512], srcp
                        )
                    else:
                        nc.scalar.activation(
                            logT_s[:, (q // 2) * 512:(q // 2 + 1) * 512],
                            srcp, AF.Copy,
                        )
                # HWDGE stores on the scalar ring whose FIFO carries only
                # these stores + the ao loads (no bulk traffic): fastest
                # store-to-trigger path; fences below keep the
                # collective-gated ao loads behind them. Even-i blocks come
                # from logT_v, odd-i from logT_s.
                dst_sp = rs_in[h][:].rearrange(
                    "(i2 two m) c -> two m i2 c", two=2, m=NMP
                )
                nc.scalar.dma_start(
                    dst_sp[0],
                    logT_v[:, h * 4 * HBS:(h + 1) * 4 * HBS]
                    .rearrange("m (i2 c) -> m i2 c", i2=4),
                )
                nc.scalar.dma_start(
                    dst_sp[1],
                    logT_s[:, h * 4 * HBS:(h + 1) * 4 * HBS]
                    .rearrange("m (i2 c) -> m i2 c", i2=4),
                )
                nc.gpsimd.collective_compute(
                    "AllToAll",
                    ALU.bypass,
                    replica_groups=[list(range(N_CORES))],
                    ins=[rs_in[h][:]],
                    outs=[rs_out[h][:]],
                )
            outt = sp.tile([128, W], F32, tag="outt")
            lgT = sp.tile([NMP, BS], F32, tag="lgT")
            for h in range(2):
                # scheduler-only fence: post-collective-h work must not be
                # reordered ahead of the stores/evacs still feeding the
                # exchanges (the scheduler models collectives as fast and
                # otherwise stalls shared engines behind AllToAll #1)
                tc.no_sync_barrier()
                ao_sb = sp.tile([N_CORES * NMP, HBS], F32, tag=f"ao{h}")
                nc.scalar.dma_start(ao_sb[:], rs_out[h][:])
                # 8-way sum on the PE: lgT_h[m, c] = sum_j ao_sb[8j+m, c]
                pss = ps.tile([128, DS], F32, tag="ps", name=f"pss{h}")
                nc.tensor.matmul(
                    pss[:NMP, :HBS], sel_sb[:], ao_sb[:],
                    start=True, stop=True,
                )
                nc.vector.tensor_copy(
                    lgT[:, h * HBS:(h + 1) * HBS], pss[:NMP, :HBS]
                )
                # ---- softmax>0.5 cond + select for this half's 4 chunks.
                # cond = (exp_m > 0.5*sum_exp); no max-subtraction needed
                # (|logit| < ~8, fp32 exp exact far beyond the 1e-5 margin).
                # out = A1 + cond*delta with masks precomputed early.
                for jb in range(4):
                    bc = 4 * h + jb
                    pl = pt.tile([128, NMP], F32, tag="pt", name=f"psl{bc}")
                    nc.tensor.transpose(
                        pl[:], lgT[:, bc * 128:(bc + 1) * 128],
                        ident[:NMP, :NMP],
                    )
                    ssum = smp.tile([128, 1], F32, tag="ssum")
                    shalf = smp.tile([128, 1], F32, tag="shalf")
                    expt = smp.tile([128, NM], F32, tag="expt")
                    u = smp.tile([128, NM], F32, tag="u")
                    tmp = smp.tile([128, GRP], F32, tag="tmp")
                    nc.scalar.activation(
                        expt[:], pl[:, :NM], AF.Exp, accum_out=ssum[:]
                    )
                    nc.vector.tensor_scalar_mul(shalf[:], ssum[:], 0.5)
                    nc.vector.tensor_scalar(
                        out=u[:], in0=expt[:], scalar1=shalf[:], scalar2=None,
                        op0=ALU.is_gt,
                    )
                    sl = slice(bc * GRP, (bc + 1) * GRP)
                    u_b = u[:].unsqueeze(-1).broadcast_to([128, NM, NE])
                    # vector only: keeps gpsimd clear for stores/triggers
                    nc.vector.tensor_tensor(
                        tmp[:].rearrange("p (m e) -> p m e", e=NE),
                        dlt[:, sl].rearrange("p (m e) -> p m e", e=NE),
                        u_b, ALU.mult,
                    )
                    nc.vector.tensor_tensor(outt[:, sl], tmp[:], a1[:, sl], ALU.add)
                nc.sync.dma_start(
                    o[:, h * W // 2:(h + 1) * W // 2],
                    outt[:, h * W // 2:(h + 1) * W // 2],
                )

    nc.compile()
    return nc


def _get_program():
    if "nc" not in _CACHE:
        _CACHE["nc"] = _build_program()
    return _CACHE["nc"]


def _const_input():
    base = (7.0 - np.arange(NE, dtype=np.float32))
    return np.ascontiguousarray(
        np.broadcast_to(np.tile(base, NBC * NM), (128, W))
    )


def kernel(pooled_hidden, Wg, Wr, rand_noise):
    global LAST_RESULTS
    ph = np.ascontiguousarray(np.asarray(pooled_hidden, dtype=np.float32))
    wg_full = np.ascontiguousarray(np.asarray(Wg, dtype=np.float32))
    wr = np.ascontiguousarray(np.asarray(Wr, dtype=np.float32))
    rn = np.ascontiguousarray(np.asarray(rand_noise, dtype=np.float32))

    nc = _get_program()
    cstv = _const_input()
    selv = np.ascontiguousarray(np.tile(np.eye(NMP, dtype=np.float32), (N_CORES, 1)))

    wrp = np.zeros((NMP, D), dtype=np.float32)
    wrp[:NM] = wr
    # wrt[p, k*8+m] = WrP[m, 128k+p]
    wrt_full = np.ascontiguousarray(
        wrp.T.reshape(NKE, 128, NMP).transpose(1, 0, 2).reshape(128, NKE * NMP)
    )
    in_maps = []
    for i in range(N_CORES):
        bsl = slice(i * BS, (i + 1) * BS)
        dsl = slice(i * DS, (i + 1) * DS)
        xT_i = np.ascontiguousarray(ph[:, dsl].T[:, _SPAN_PERM])   # [512, 8192]
        wg_i = np.ascontiguousarray(wg_full[:, dsl])               # [4096, 512]
        # nz[p, c*56 + m*8 + e] = rn[m, 1024*i + 128*c + p, e]
        nz_i = np.ascontiguousarray(
            rn[:, bsl, :].transpose(1, 0, 2)
            .reshape(NBC, 128, GRP).transpose(1, 0, 2).reshape(128, W)
        )
        in_maps.append(
            {"xT": xT_i, "wg": wg_i, "wrt": wrt_full, "nz": nz_i, "cst": cstv,
             "sel": selv}
        )

    res = run_bass_kernel_spmd(nc, in_maps, list(range(N_CORES)))
    LAST_RESULTS = res

    out = np.empty((NM, B, NE), dtype=np.float32)
    for i, r in enumerate(res.results):
        oc = r["o"]  # [128, 448]
        out[:, i * BS:(i + 1) * BS, :] = (
            oc.reshape(128, NBC, NM, NE).transpose(2, 1, 0, 3).reshape(NM, BS, NE)
        )
    return out
